# revision 7
# baseline (speedup 1.0000x reference)
"""GCN layer kernel for Trainium2 (Bass/Tile), data-parallel over batch.

Reference computation (per batch element):
    deg = A.sum(-1); d = deg ** -0.5
    t   = X @ W.T + b
    out = relu(diag(d) @ A @ diag(d) @ t)

Per-core mapping (8 cores, one batch element each):
  - A streams in as 16 row-tiles [128, 2048], cast f32->bf16 during the DMA
    (SWDGE compute-DMA), then transposed SBUF->SBUF by the DMA xbar
    (2-byte-only path, ~90% of DMA bandwidth) into an 8 MB bf16 store at_big.
    The tensor engine contracts over partitions, so A's contraction index
    (its column) must live on partitions; the xbar does that off the PE.
    Xbar layout: out[p, 16r + b] = in[r, 128b + p], so the matmul stationary
    for chunk (k-tile b, mu) is a stride-16 AP - no repacking needed.
  - Row degrees reduce on DVE (2x bf16 rate) from the natural bf16 tiles;
    d = sqrt(1/deg) via DVE reciprocal + ACT sqrt.
  - t = X @ W.T in bf16 (X tiles also xbar-transposed; W.T passed
    pre-transposed from host as a layout choice); bias added in f32 from a
    broadcast tile; y = d * t rounded to bf16 by the ACT scale pass.
  - Main matmul accumulates out[mu] = sum_k AT(k,mu).T @ y[k] in PSUM f32:
    8 accumulator banks run during the stream (triangular schedule: product
    (k, mu) is runnable once row-tiles k and mu have both arrived), the
    remaining 8 row-tiles run as a tail batch afterwards.
  - Drain: relu(d * psum) on ACT, then DMA out (f32).
"""

from contextlib import ExitStack

import numpy as np

import concourse.bacc as bacc
import concourse.mybir as mybir
import concourse.tile as tile
from concourse.bass_utils import run_bass_kernel_spmd

B = 8
N = 2048
F = 256
P = 128
NT = N // P  # 16 row tiles
FT = F // P  # 2 feature tiles
F32 = mybir.dt.float32
BF16 = mybir.dt.bfloat16
COPY = mybir.ActivationFunctionType.Copy
RELU = mybir.ActivationFunctionType.Relu
ACC_SLOTS = 8  # PSUM banks used as out accumulators


def _emit(ctx: ExitStack, tc: tile.TileContext, A, X, WT, BIAS, OUT):
    nc = tc.nc

    const = ctx.enter_context(tc.tile_pool(name="const", bufs=1))
    stage = ctx.enter_context(tc.tile_pool(name="stage", bufs=3))
    xstage = ctx.enter_context(tc.tile_pool(name="xstage", bufs=2))
    xt_pool = ctx.enter_context(tc.tile_pool(name="xt", bufs=3))
    at_pool = ctx.enter_context(tc.tile_pool(name="at", bufs=1))
    outstage = ctx.enter_context(tc.tile_pool(name="outstage", bufs=4))
    psum_acc = ctx.enter_context(
        tc.tile_pool(name="psum_acc", bufs=ACC_SLOTS, space="PSUM")
    )

    # W.T resident in SBUF as bf16 (cast during SWDGE DMA)
    wt_sb = const.tile([P, FT * F], BF16, tag="wt")
    for phi in range(FT):
        nc.gpsimd.dma_start(
            out=wt_sb[:, phi * F : (phi + 1) * F], in_=WT[phi * P : (phi + 1) * P, :]
        )

    # bias broadcast tile [128, 256] f32 built via ones-column outer product
    b_row = const.tile([1, F], F32, tag="brow")
    nc.sync.dma_start(out=b_row[:, :], in_=BIAS[:, :])
    ones_row = const.tile([1, P], F32, tag="ones")
    nc.vector.memset(ones_row[:, :], 1.0)
    b_psum = psum_acc.tile([P, F], F32, tag="acc", name="b_psum")
    nc.tensor.matmul(b_psum[:, :], ones_row[:, :], b_row[:, :], start=True, stop=True)
    b_bcast = const.tile([P, F], F32, tag="bbc")
    nc.scalar.copy(b_bcast[:, :], b_psum[:, :])

    # degree -> d = sqrt(1/deg) storage, one column per row-tile
    deg = const.tile([P, NT], F32, tag="deg")
    rec = const.tile([P, NT], F32, tag="rec")
    dinv = const.tile([P, NT], F32, tag="dinv")

    # t = X W^T + b in f32; y = bf16 rounded d*t
    t_big = const.tile([P, NT * F], F32, tag="t")
    y_big = const.tile([P, NT * F], BF16, tag="y")

    # transposed adjacency store (xbar 3D-out layout), tile mu at [:, 2048*mu:]:
    # at_big[p, 2048*mu + 128*k + r] = A[128*mu + r, 128*k + p]
    at_big = at_pool.tile([P, NT * N], BF16, tag="at")
    # view [p, mu, k, r]: stationary chunk (k, mu) = at_view[:, mu, k, :] (contiguous)
    at_view = at_big[:, :].rearrange("p (m e r) -> p m e r", m=NT, e=NT)

    # ---- t = X @ W.T + b (per row-tile), X chunks transposed by the xbar ----
    for mu in range(NT):
        xs = xstage.tile([P, F], BF16, tag="xs")
        nc.gpsimd.dma_start(out=xs[:, :], in_=X[mu * P : (mu + 1) * P, :])
        # xt[p, 128*phi + r] = X[128*mu + r, 128*phi + p]
        xt = xt_pool.tile([P, F], BF16, tag="xt")
        nc.sync.dma_start_transpose(
            xt[:, :].rearrange("p (phi r) -> p phi r", phi=FT), xs[:, :]
        )
        xt_v = xt[:, :].rearrange("p (phi r) -> p phi r", phi=FT)
        t_psum = psum_acc.tile([P, F], F32, tag="acc", name="t_psum")
        for phi in range(FT):
            nc.tensor.matmul(
                t_psum[:, :],
                xt_v[:, phi, :],
                wt_sb[:, phi * F : (phi + 1) * F],
                start=(phi == 0),
                stop=(phi == FT - 1),
            )
        # t + b -> t_big f32 (DVE, PSUM read)
        nc.vector.tensor_add(t_big[:, mu * F : (mu + 1) * F], t_psum[:, :], b_bcast[:, :])

    # ---- stream A row-tiles: degree, d, y, xbar transpose, main matmul ----
    acc_tiles = {}

    def emit_product(k, mu):
        nc.tensor.matmul(
            acc_tiles[mu][:, :],
            at_view[:, mu, k, :],
            y_big[:, k * F : (k + 1) * F],
            start=(k == 0),
            stop=(k == NT - 1),
        )

    def emit_drain(mu):
        os = outstage.tile([P, F], F32, tag="os")
        nc.scalar.activation(
            os[:, :], acc_tiles[mu][:, :], RELU, scale=dinv[:, mu : mu + 1]
        )
        nc.sync.dma_start(out=OUT[mu * P : (mu + 1) * P, :], in_=os[:, :])

    for i in range(NT):
        a_bf = stage.tile([P, N], BF16, tag="a")
        nc.gpsimd.dma_start(out=a_bf[:, :], in_=A[i * P : (i + 1) * P, :])
        # degree of these 128 rows (bf16 input, f32 accumulate); d = sqrt(1/deg)
        nc.vector.tensor_reduce(
            deg[:, i : i + 1],
            a_bf[:, :],
            axis=mybir.AxisListType.X,
            op=mybir.AluOpType.add,
        )
        nc.vector.reciprocal(rec[:, i : i + 1], deg[:, i : i + 1])
        nc.scalar.sqrt(dinv[:, i : i + 1], rec[:, i : i + 1])
        # y[i] = d[i] * t[i], rounded to bf16
        nc.scalar.activation(
            y_big[:, i * F : (i + 1) * F],
            t_big[:, i * F : (i + 1) * F],
            COPY,
            scale=dinv[:, i : i + 1],
        )
        # xbar-transpose this row-tile into at_big (3D out: chunk-contiguous)
        nc.sync.dma_start_transpose(at_view[:, i, :, :], a_bf[:, :])
        # main-matmul products that just became runnable (early accumulators):
        # every (k, mu) pair with max(k, mu) == i and mu < ACC_SLOTS
        if i < ACC_SLOTS:
            acc_tiles[i] = psum_acc.tile([P, F], F32, tag="acc", name=f"acc_{i}")
            for k in range(i + 1):
                emit_product(k, i)
        for mu in range(min(i, ACC_SLOTS)):
            emit_product(i, mu)

    # ---- drains + tail batches ----
    for mu in range(ACC_SLOTS):
        emit_drain(mu)
    for mu in range(ACC_SLOTS, NT):
        acc_tiles[mu] = psum_acc.tile([P, F], F32, tag="acc", name=f"acc_{mu}")
        for k in range(NT):
            emit_product(k, mu)
        emit_drain(mu)


_cached_nc = None


def _build():
    nc = bacc.Bacc("TRN2", target_bir_lowering=False, debug=False)
    A = nc.dram_tensor("adj", [N, N], F32, kind="ExternalInput").ap()
    X = nc.dram_tensor("x", [N, F], F32, kind="ExternalInput").ap()
    WT = nc.dram_tensor("wt", [F, F], F32, kind="ExternalInput").ap()
    BIAS = nc.dram_tensor("bias", [1, F], F32, kind="ExternalInput").ap()
    OUT = nc.dram_tensor("out", [N, F], F32, kind="ExternalOutput").ap()
    with tile.TileContext(nc) as tc:
        with ExitStack() as ctx:
            _emit(ctx, tc, A, X, WT, BIAS, OUT)
    nc.compile()
    return nc


def get_nc():
    global _cached_nc
    if _cached_nc is None:
        _cached_nc = _build()
    return _cached_nc


def make_in_maps(node_features, adj_matrix, W, b):
    node_features = np.asarray(node_features, dtype=np.float32)
    adj_matrix = np.asarray(adj_matrix, dtype=np.float32)
    wt = np.ascontiguousarray(np.asarray(W, dtype=np.float32).T)
    bias = np.ascontiguousarray(np.asarray(b, dtype=np.float32).reshape(1, F))
    return [
        {
            "adj": np.ascontiguousarray(adj_matrix[c]),
            "x": np.ascontiguousarray(node_features[c]),
            "wt": wt,
            "bias": bias,
        }
        for c in range(B)
    ]


def kernel(node_features, adj_matrix, W, b):
    nc = get_nc()
    in_maps = make_in_maps(node_features, adj_matrix, W, b)
    res = run_bass_kernel_spmd(nc, in_maps, core_ids=list(range(B)))
    return np.stack([r["out"] for r in res.results], axis=0)


# revision 8
# speedup vs baseline: 1.1279x; 1.1279x over previous
"""GCN layer kernel for Trainium2 (Bass/Tile), data-parallel over batch.

Reference computation (per batch element):
    deg = A.sum(-1); d = deg ** -0.5
    t   = X @ W.T + b
    out = relu(diag(d) @ A @ diag(d) @ t)

Per-core mapping (8 cores, one batch element each):
  - A streams in as 16 row-tiles [128, 2048] (HWDGE f32 loads), cast
    f32->bf16 on GpSimd (1-input ops run at line rate there), then transposed
    SBUF->SBUF by the DMA xbar (2-byte-only path) into an 8 MB bf16 at_big.
    The tensor engine contracts over partitions, so A's contraction index
    (its column) must live on partitions; the xbar does that off the PE.
    Xbar layout: out[p, 16r + b] = in[r, 128b + p], so the matmul stationary
    for chunk (k-tile b, mu) is a stride-16 AP - no repacking needed.
  - Row degrees reduce on DVE (2x bf16 rate) from the natural bf16 tiles;
    d = sqrt(1/deg) via DVE reciprocal + ACT sqrt.
  - t = X @ W.T in bf16 (X tiles also xbar-transposed; W.T passed
    pre-transposed from host as a layout choice); bias added in f32 from a
    broadcast tile; y = d * t rounded to bf16 by the ACT scale pass.
  - Main matmul accumulates out[mu] = sum_k AT(k,mu).T @ y[k] in PSUM f32:
    8 accumulator banks run during the stream (triangular schedule: product
    (k, mu) is runnable once row-tiles k and mu have both arrived), the
    remaining 8 row-tiles run as a tail batch afterwards.
  - Drain: relu(d * psum) on ACT, then DMA out (f32).
"""

from contextlib import ExitStack

import numpy as np

import concourse.bacc as bacc
import concourse.mybir as mybir
import concourse.tile as tile
from concourse.bass_utils import run_bass_kernel_spmd

B = 8
N = 2048
F = 256
P = 128
NT = N // P  # 16 row tiles
FT = F // P  # 2 feature tiles
F32 = mybir.dt.float32
BF16 = mybir.dt.bfloat16
COPY = mybir.ActivationFunctionType.Copy
RELU = mybir.ActivationFunctionType.Relu
ACC_SLOTS = 8  # PSUM banks used as out accumulators


def _emit(ctx: ExitStack, tc: tile.TileContext, A, X, WT, BIAS, OUT):
    nc = tc.nc

    const = ctx.enter_context(tc.tile_pool(name="const", bufs=1))
    stage = ctx.enter_context(tc.tile_pool(name="stage", bufs=3))
    xstage = ctx.enter_context(tc.tile_pool(name="xstage", bufs=2))
    xt_pool = ctx.enter_context(tc.tile_pool(name="xt", bufs=3))
    at_pool = ctx.enter_context(tc.tile_pool(name="at", bufs=1))
    outstage = ctx.enter_context(tc.tile_pool(name="outstage", bufs=4))
    psum_acc = ctx.enter_context(
        tc.tile_pool(name="psum_acc", bufs=ACC_SLOTS, space="PSUM")
    )

    # W.T resident in SBUF as bf16 (f32 HWDGE load + Pool cast)
    wt_stage = const.tile([P, FT * F], F32, tag="wts")
    for phi in range(FT):
        nc.sync.dma_start(
            out=wt_stage[:, phi * F : (phi + 1) * F], in_=WT[phi * P : (phi + 1) * P, :]
        )
    wt_sb = const.tile([P, FT * F], BF16, tag="wt")
    nc.gpsimd.tensor_copy(wt_sb[:, :], wt_stage[:, :])

    # bias broadcast tile [128, 256] f32 built via ones-column outer product
    b_row = const.tile([1, F], F32, tag="brow")
    nc.sync.dma_start(out=b_row[:, :], in_=BIAS[:, :])
    ones_row = const.tile([1, P], F32, tag="ones")
    nc.vector.memset(ones_row[:, :], 1.0)
    b_psum = psum_acc.tile([P, F], F32, tag="acc", name="b_psum")
    nc.tensor.matmul(b_psum[:, :], ones_row[:, :], b_row[:, :], start=True, stop=True)
    b_bcast = const.tile([P, F], F32, tag="bbc")
    nc.scalar.copy(b_bcast[:, :], b_psum[:, :])

    # degree -> d = sqrt(1/deg) storage, one column per row-tile
    deg = const.tile([P, NT], F32, tag="deg")
    rec = const.tile([P, NT], F32, tag="rec")
    dinv = const.tile([P, NT], F32, tag="dinv")

    # t = X W^T + b in f32; y = bf16 rounded d*t
    t_big = const.tile([P, NT * F], F32, tag="t")
    y_big = const.tile([P, NT * F], BF16, tag="y")

    # transposed adjacency store (xbar 3D-out layout), tile mu at [:, 2048*mu:]:
    # at_big[p, 2048*mu + 128*k + r] = A[128*mu + r, 128*k + p]
    at_big = at_pool.tile([P, NT * N], BF16, tag="at")
    # view [p, mu, k, r]: stationary chunk (k, mu) = at_view[:, mu, k, :] (contiguous)
    at_view = at_big[:, :].rearrange("p (m e r) -> p m e r", m=NT, e=NT)

    # ---- t = X @ W.T + b (per row-tile), X chunks transposed by the xbar ----
    for mu in range(NT):
        xf = xstage.tile([P, F], F32, tag="xf")
        nc.scalar.dma_start(out=xf[:, :], in_=X[mu * P : (mu + 1) * P, :])
        xs = xstage.tile([P, F], BF16, tag="xs")
        nc.gpsimd.tensor_copy(xs[:, :], xf[:, :])
        # xt[p, 128*phi + r] = X[128*mu + r, 128*phi + p]
        xt = xt_pool.tile([P, F], BF16, tag="xt")
        nc.sync.dma_start_transpose(
            xt[:, :].rearrange("p (phi r) -> p phi r", phi=FT), xs[:, :]
        )
        xt_v = xt[:, :].rearrange("p (phi r) -> p phi r", phi=FT)
        t_psum = psum_acc.tile([P, F], F32, tag="acc", name="t_psum")
        for phi in range(FT):
            nc.tensor.matmul(
                t_psum[:, :],
                xt_v[:, phi, :],
                wt_sb[:, phi * F : (phi + 1) * F],
                start=(phi == 0),
                stop=(phi == FT - 1),
            )
        # t + b -> t_big f32 (DVE, PSUM read)
        nc.vector.tensor_add(t_big[:, mu * F : (mu + 1) * F], t_psum[:, :], b_bcast[:, :])

    # ---- stream A row-tiles: degree, d, y, xbar transpose, main matmul ----
    acc_tiles = {}

    def emit_product(k, mu):
        nc.tensor.matmul(
            acc_tiles[mu][:, :],
            at_view[:, mu, k, :],
            y_big[:, k * F : (k + 1) * F],
            start=(k == 0),
            stop=(k == NT - 1),
        )

    def emit_drain(mu):
        os = outstage.tile([P, F], F32, tag="os")
        nc.scalar.activation(
            os[:, :], acc_tiles[mu][:, :], RELU, scale=dinv[:, mu : mu + 1]
        )
        nc.scalar.dma_start(out=OUT[mu * P : (mu + 1) * P, :], in_=os[:, :])

    for i in range(NT):
        a_f32 = stage.tile([P, N], F32, tag="af")
        nc.sync.dma_start(out=a_f32[:, :], in_=A[i * P : (i + 1) * P, :])
        a_bf = stage.tile([P, N], BF16, tag="a")
        nc.gpsimd.tensor_copy(a_bf[:, :], a_f32[:, :])
        # degree of these 128 rows (bf16 input, f32 accumulate); d = sqrt(1/deg)
        nc.vector.tensor_reduce(
            deg[:, i : i + 1],
            a_bf[:, :],
            axis=mybir.AxisListType.X,
            op=mybir.AluOpType.add,
        )
        nc.vector.reciprocal(rec[:, i : i + 1], deg[:, i : i + 1])
        nc.scalar.sqrt(dinv[:, i : i + 1], rec[:, i : i + 1])
        # y[i] = d[i] * t[i], rounded to bf16
        nc.scalar.activation(
            y_big[:, i * F : (i + 1) * F],
            t_big[:, i * F : (i + 1) * F],
            COPY,
            scale=dinv[:, i : i + 1],
        )
        # xbar-transpose this row-tile into at_big (3D out: chunk-contiguous)
        nc.sync.dma_start_transpose(at_view[:, i, :, :], a_bf[:, :])
        # main-matmul products that just became runnable (early accumulators):
        # every (k, mu) pair with max(k, mu) == i and mu < ACC_SLOTS
        if i < ACC_SLOTS:
            acc_tiles[i] = psum_acc.tile([P, F], F32, tag="acc", name=f"acc_{i}")
            for k in range(i + 1):
                emit_product(k, i)
        for mu in range(min(i, ACC_SLOTS)):
            emit_product(i, mu)

    # ---- drains + tail batches ----
    for mu in range(ACC_SLOTS):
        emit_drain(mu)
    for mu in range(ACC_SLOTS, NT):
        acc_tiles[mu] = psum_acc.tile([P, F], F32, tag="acc", name=f"acc_{mu}")
        for k in range(NT):
            emit_product(k, mu)
        emit_drain(mu)


_cached_nc = None


def _build():
    nc = bacc.Bacc("TRN2", target_bir_lowering=False, debug=False)
    A = nc.dram_tensor("adj", [N, N], F32, kind="ExternalInput").ap()
    X = nc.dram_tensor("x", [N, F], F32, kind="ExternalInput").ap()
    WT = nc.dram_tensor("wt", [F, F], F32, kind="ExternalInput").ap()
    BIAS = nc.dram_tensor("bias", [1, F], F32, kind="ExternalInput").ap()
    OUT = nc.dram_tensor("out", [N, F], F32, kind="ExternalOutput").ap()
    with tile.TileContext(nc) as tc:
        with ExitStack() as ctx:
            _emit(ctx, tc, A, X, WT, BIAS, OUT)
    nc.compile()
    return nc


def get_nc():
    global _cached_nc
    if _cached_nc is None:
        _cached_nc = _build()
    return _cached_nc


def make_in_maps(node_features, adj_matrix, W, b):
    node_features = np.asarray(node_features, dtype=np.float32)
    adj_matrix = np.asarray(adj_matrix, dtype=np.float32)
    wt = np.ascontiguousarray(np.asarray(W, dtype=np.float32).T)
    bias = np.ascontiguousarray(np.asarray(b, dtype=np.float32).reshape(1, F))
    return [
        {
            "adj": np.ascontiguousarray(adj_matrix[c]),
            "x": np.ascontiguousarray(node_features[c]),
            "wt": wt,
            "bias": bias,
        }
        for c in range(B)
    ]


def kernel(node_features, adj_matrix, W, b):
    nc = get_nc()
    in_maps = make_in_maps(node_features, adj_matrix, W, b)
    res = run_bass_kernel_spmd(nc, in_maps, core_ids=list(range(B)))
    return np.stack([r["out"] for r in res.results], axis=0)


# revision 9
# speedup vs baseline: 1.1862x; 1.0517x over previous
"""GCN layer kernel for Trainium2 (Bass/Tile), data-parallel over batch.

Reference computation (per batch element):
    deg = A.sum(-1); d = deg ** -0.5
    t   = X @ W.T + b
    out = relu(diag(d) @ A @ diag(d) @ t)

Per-core mapping (8 cores, one batch element each):
  - A streams in as 16 row-tiles [128, 2048] (HWDGE f32 loads), cast
    f32->bf16 on GpSimd (1-input ops run at line rate there), then transposed
    SBUF->SBUF by the DMA xbar (2-byte-only path) into an 8 MB bf16 at_big.
    The tensor engine contracts over partitions, so A's contraction index
    (its column) must live on partitions; the xbar does that off the PE.
    Xbar layout: out[p, 16r + b] = in[r, 128b + p], so the matmul stationary
    for chunk (k-tile b, mu) is a stride-16 AP - no repacking needed.
  - Row degrees reduce on DVE (2x bf16 rate) from the natural bf16 tiles;
    d = sqrt(1/deg) via DVE reciprocal + ACT sqrt.
  - t = X @ W.T in bf16 (X tiles also xbar-transposed; W.T passed
    pre-transposed from host as a layout choice); bias added in f32 from a
    broadcast tile; y = d * t rounded to bf16 by the ACT scale pass.
  - Main matmul accumulates out[mu] = sum_k AT(k,mu).T @ y[k] in PSUM f32:
    8 accumulator banks run during the stream (triangular schedule: product
    (k, mu) is runnable once row-tiles k and mu have both arrived), the
    remaining 8 row-tiles run as a tail batch afterwards.
  - Drain: relu(d * psum) on ACT, then DMA out (f32).
"""

from contextlib import ExitStack

import numpy as np

import concourse.bacc as bacc
import concourse.mybir as mybir
import concourse.tile as tile
from concourse.bass_utils import run_bass_kernel_spmd

B = 8
N = 2048
F = 256
P = 128
NT = N // P  # 16 row tiles
FT = F // P  # 2 feature tiles
F32 = mybir.dt.float32
BF16 = mybir.dt.bfloat16
COPY = mybir.ActivationFunctionType.Copy
RELU = mybir.ActivationFunctionType.Relu
ACC_SLOTS = 8  # PSUM banks used as out accumulators


def _emit(ctx: ExitStack, tc: tile.TileContext, A, X, WT, BIAS, OUT):
    nc = tc.nc

    const = ctx.enter_context(tc.tile_pool(name="const", bufs=1))
    stage = ctx.enter_context(tc.tile_pool(name="stage", bufs=3))
    xstage = ctx.enter_context(tc.tile_pool(name="xstage", bufs=2))
    xt_pool = ctx.enter_context(tc.tile_pool(name="xt", bufs=3))
    at_pool = ctx.enter_context(tc.tile_pool(name="at", bufs=1))
    outstage = ctx.enter_context(tc.tile_pool(name="outstage", bufs=4))
    psum_acc = ctx.enter_context(
        tc.tile_pool(name="psum_acc", bufs=ACC_SLOTS, space="PSUM")
    )

    # W.T resident in SBUF as bf16 (f32 HWDGE load + Pool cast)
    wt_stage = const.tile([P, FT * F], F32, tag="wts")
    for phi in range(FT):
        nc.sync.dma_start(
            out=wt_stage[:, phi * F : (phi + 1) * F], in_=WT[phi * P : (phi + 1) * P, :]
        )
    wt_sb = const.tile([P, FT * F], BF16, tag="wt")
    nc.gpsimd.tensor_copy(wt_sb[:, :], wt_stage[:, :])

    # bias broadcast tile [128, 256] f32 built via ones-column outer product
    b_row = const.tile([1, F], F32, tag="brow")
    nc.sync.dma_start(out=b_row[:, :], in_=BIAS[:, :])
    ones_row = const.tile([1, P], F32, tag="ones")
    nc.vector.memset(ones_row[:, :], 1.0)
    b_psum = psum_acc.tile([P, F], F32, tag="acc", name="b_psum")
    nc.tensor.matmul(b_psum[:, :], ones_row[:, :], b_row[:, :], start=True, stop=True)
    b_bcast = const.tile([P, F], F32, tag="bbc")
    nc.scalar.copy(b_bcast[:, :], b_psum[:, :])

    # degree -> d = sqrt(1/deg) storage, one column per row-tile
    deg = const.tile([P, NT], F32, tag="deg")
    rec = const.tile([P, NT], F32, tag="rec")
    dinv = const.tile([P, NT], F32, tag="dinv")

    # t = X W^T + b in f32; y = bf16 rounded d*t
    t_big = const.tile([P, NT * F], F32, tag="t")
    y_big = const.tile([P, NT * F], BF16, tag="y")

    # transposed adjacency store (xbar 3D-out layout), tile mu at [:, 2048*mu:]:
    # at_big[p, 2048*mu + 128*k + r] = A[128*mu + r, 128*k + p]
    at_big = at_pool.tile([P, NT * N], BF16, tag="at")
    # view [p, mu, k, r]: stationary chunk (k, mu) = at_view[:, mu, k, :] (contiguous)
    at_view = at_big[:, :].rearrange("p (m e r) -> p m e r", m=NT, e=NT)

    # ---- t = X @ W.T + b (per row-tile), X chunks transposed by the xbar ----
    for mu in range(NT):
        xf = xstage.tile([P, F], F32, tag="xf")
        nc.scalar.dma_start(out=xf[:, :], in_=X[mu * P : (mu + 1) * P, :])
        xs = xstage.tile([P, F], BF16, tag="xs")
        nc.gpsimd.tensor_copy(xs[:, :], xf[:, :])
        # xt[p, 128*phi + r] = X[128*mu + r, 128*phi + p]
        xt = xt_pool.tile([P, F], BF16, tag="xt")
        nc.sync.dma_start_transpose(
            xt[:, :].rearrange("p (phi r) -> p phi r", phi=FT), xs[:, :]
        )
        xt_v = xt[:, :].rearrange("p (phi r) -> p phi r", phi=FT)
        t_psum = psum_acc.tile([P, F], F32, tag="acc", name="t_psum")
        for phi in range(FT):
            nc.tensor.matmul(
                t_psum[:, :],
                xt_v[:, phi, :],
                wt_sb[:, phi * F : (phi + 1) * F],
                start=(phi == 0),
                stop=(phi == FT - 1),
            )
        # t + b -> t_big f32 (DVE, PSUM read)
        nc.vector.tensor_add(t_big[:, mu * F : (mu + 1) * F], t_psum[:, :], b_bcast[:, :])

    # ---- stream A row-tiles: degree, d, y, xbar transpose, main matmul ----
    acc_tiles = {}

    def emit_product(k, mu):
        nc.tensor.matmul(
            acc_tiles[mu][:, :],
            at_view[:, mu, k, :],
            y_big[:, k * F : (k + 1) * F],
            start=(k == 0),
            stop=(k == NT - 1),
        )

    def emit_drain(mu):
        os = outstage.tile([P, F], F32, tag="os")
        nc.scalar.activation(
            os[:, :], acc_tiles[mu][:, :], RELU, scale=dinv[:, mu : mu + 1]
        )
        nc.gpsimd.dma_start(out=OUT[mu * P : (mu + 1) * P, :], in_=os[:, :])

    for i in range(NT):
        a_f32 = stage.tile([P, N], F32, tag="af")
        nc.sync.dma_start(out=a_f32[:, :], in_=A[i * P : (i + 1) * P, :])
        # one DVE pass: bf16 cast (matmul operand) + row-sum degree accumulator
        a_bf = stage.tile([P, N], BF16, tag="a")
        nc.vector.tensor_scalar(
            out=a_bf[:, :],
            in0=a_f32[:, :],
            scalar1=0.0,
            scalar2=None,
            op0=mybir.AluOpType.add,
            op1=mybir.AluOpType.add,
            accum_out=deg[:, i : i + 1],
        )
        nc.vector.reciprocal(rec[:, i : i + 1], deg[:, i : i + 1])
        nc.scalar.sqrt(dinv[:, i : i + 1], rec[:, i : i + 1])
        # y[i] = d[i] * t[i], rounded to bf16
        nc.scalar.activation(
            y_big[:, i * F : (i + 1) * F],
            t_big[:, i * F : (i + 1) * F],
            COPY,
            scale=dinv[:, i : i + 1],
        )
        # xbar-transpose this row-tile into at_big (3D out: chunk-contiguous)
        nc.sync.dma_start_transpose(at_view[:, i, :, :], a_bf[:, :])
        # main-matmul products that just became runnable (early accumulators):
        # every (k, mu) pair with max(k, mu) == i and mu < ACC_SLOTS
        if i < ACC_SLOTS:
            acc_tiles[i] = psum_acc.tile([P, F], F32, tag="acc", name=f"acc_{i}")
            for k in range(i + 1):
                emit_product(k, i)
        for mu in range(min(i, ACC_SLOTS)):
            emit_product(i, mu)

    # ---- drains + tail batches ----
    for mu in range(ACC_SLOTS):
        emit_drain(mu)
    for mu in range(ACC_SLOTS, NT):
        acc_tiles[mu] = psum_acc.tile([P, F], F32, tag="acc", name=f"acc_{mu}")
        for k in range(NT):
            emit_product(k, mu)
        emit_drain(mu)


_cached_nc = None


def _build():
    nc = bacc.Bacc("TRN2", target_bir_lowering=False, debug=False)
    A = nc.dram_tensor("adj", [N, N], F32, kind="ExternalInput").ap()
    X = nc.dram_tensor("x", [N, F], F32, kind="ExternalInput").ap()
    WT = nc.dram_tensor("wt", [F, F], F32, kind="ExternalInput").ap()
    BIAS = nc.dram_tensor("bias", [1, F], F32, kind="ExternalInput").ap()
    OUT = nc.dram_tensor("out", [N, F], F32, kind="ExternalOutput").ap()
    with tile.TileContext(nc) as tc:
        with ExitStack() as ctx:
            _emit(ctx, tc, A, X, WT, BIAS, OUT)
    nc.compile()
    return nc


def get_nc():
    global _cached_nc
    if _cached_nc is None:
        _cached_nc = _build()
    return _cached_nc


def make_in_maps(node_features, adj_matrix, W, b):
    node_features = np.asarray(node_features, dtype=np.float32)
    adj_matrix = np.asarray(adj_matrix, dtype=np.float32)
    wt = np.ascontiguousarray(np.asarray(W, dtype=np.float32).T)
    bias = np.ascontiguousarray(np.asarray(b, dtype=np.float32).reshape(1, F))
    return [
        {
            "adj": np.ascontiguousarray(adj_matrix[c]),
            "x": np.ascontiguousarray(node_features[c]),
            "wt": wt,
            "bias": bias,
        }
        for c in range(B)
    ]


def kernel(node_features, adj_matrix, W, b):
    nc = get_nc()
    in_maps = make_in_maps(node_features, adj_matrix, W, b)
    res = run_bass_kernel_spmd(nc, in_maps, core_ids=list(range(B)))
    return np.stack([r["out"] for r in res.results], axis=0)


# revision 10
# speedup vs baseline: 1.2145x; 1.0238x over previous
"""GCN layer kernel for Trainium2 (Bass/Tile), data-parallel over batch.

Reference computation (per batch element):
    deg = A.sum(-1); d = deg ** -0.5
    t   = X @ W.T + b
    out = relu(diag(d) @ A @ diag(d) @ t)

Per-core mapping (8 cores, one batch element each):
  - A streams in as 16 row-tiles [128, 2048] (HWDGE f32 loads), cast
    f32->bf16 on GpSimd (1-input ops run at line rate there), then transposed
    SBUF->SBUF by the DMA xbar (2-byte-only path) into an 8 MB bf16 at_big.
    The tensor engine contracts over partitions, so A's contraction index
    (its column) must live on partitions; the xbar does that off the PE.
    Xbar layout: out[p, 16r + b] = in[r, 128b + p], so the matmul stationary
    for chunk (k-tile b, mu) is a stride-16 AP - no repacking needed.
  - Row degrees reduce on DVE (2x bf16 rate) from the natural bf16 tiles;
    d = sqrt(1/deg) via DVE reciprocal + ACT sqrt.
  - t = X @ W.T in bf16 (X tiles also xbar-transposed; W.T passed
    pre-transposed from host as a layout choice); bias added in f32 from a
    broadcast tile; y = d * t rounded to bf16 by the ACT scale pass.
  - Main matmul accumulates out[mu] = sum_k AT(k,mu).T @ y[k] in PSUM f32:
    8 accumulator banks run during the stream (triangular schedule: product
    (k, mu) is runnable once row-tiles k and mu have both arrived), the
    remaining 8 row-tiles run as a tail batch afterwards.
  - Drain: relu(d * psum) on ACT, then DMA out (f32).
"""

from contextlib import ExitStack

import numpy as np

import concourse.bacc as bacc
import concourse.mybir as mybir
import concourse.tile as tile
from concourse.bass_utils import run_bass_kernel_spmd

B = 8
N = 2048
F = 256
P = 128
NT = N // P  # 16 row tiles
FT = F // P  # 2 feature tiles
F32 = mybir.dt.float32
BF16 = mybir.dt.bfloat16
COPY = mybir.ActivationFunctionType.Copy
RELU = mybir.ActivationFunctionType.Relu
ACC_SLOTS = 8  # PSUM banks used as out accumulators


def _emit(ctx: ExitStack, tc: tile.TileContext, A, X, WT, BIAS, OUT):
    nc = tc.nc

    const = ctx.enter_context(tc.tile_pool(name="const", bufs=1))
    stage = ctx.enter_context(tc.tile_pool(name="stage", bufs=4))
    at_pool = ctx.enter_context(tc.tile_pool(name="at", bufs=1))
    outstage = ctx.enter_context(tc.tile_pool(name="outstage", bufs=4))
    psum_acc = ctx.enter_context(
        tc.tile_pool(name="psum_acc", bufs=ACC_SLOTS, space="PSUM")
    )

    # W.T resident in SBUF as bf16 (f32 HWDGE load + Pool cast)
    wt_stage = const.tile([P, FT * F], F32, tag="wts")
    for phi in range(FT):
        nc.sync.dma_start(
            out=wt_stage[:, phi * F : (phi + 1) * F], in_=WT[phi * P : (phi + 1) * P, :]
        )
    wt_sb = const.tile([P, FT * F], BF16, tag="wt")
    nc.gpsimd.tensor_copy(wt_sb[:, :], wt_stage[:, :])

    # bias broadcast tile [128, 256] f32 built via ones-column outer product
    b_row = const.tile([1, F], F32, tag="brow")
    nc.sync.dma_start(out=b_row[:, :], in_=BIAS[:, :])
    ones_row = const.tile([1, P], F32, tag="ones")
    nc.vector.memset(ones_row[:, :], 1.0)
    b_psum = psum_acc.tile([P, F], F32, tag="acc", name="b_psum")
    nc.tensor.matmul(b_psum[:, :], ones_row[:, :], b_row[:, :], start=True, stop=True)
    b_bcast = const.tile([P, F], F32, tag="bbc")
    nc.scalar.copy(b_bcast[:, :], b_psum[:, :])

    # degree -> d = sqrt(1/deg) storage, one column per row-tile
    deg = const.tile([P, NT], F32, tag="deg")
    rec = const.tile([P, NT], F32, tag="rec")
    dinv = const.tile([P, NT], F32, tag="dinv")

    # t = X W^T + b in f32; y = bf16 rounded d*t
    t_big = const.tile([P, NT * F], F32, tag="t")
    y_big = const.tile([P, NT * F], BF16, tag="y")

    # transposed adjacency store (xbar 3D-out layout), tile mu at [:, 2048*mu:]:
    # at_big[p, 2048*mu + 128*k + r] = A[128*mu + r, 128*k + p]
    at_big = at_pool.tile([P, NT * N], BF16, tag="at")
    # view [p, mu, k, r]: stationary chunk (k, mu) = at_view[:, mu, k, :] (contiguous)
    at_view = at_big[:, :].rearrange("p (m e r) -> p m e r", m=NT, e=NT)

    # ---- t = X @ W.T + b: bulk-load X, Pool-cast to bf16, one xbar call ----
    xs_f32 = const.tile([P, NT * F], F32, tag="xsf")
    xs_bf = const.tile([P, NT * F], BF16, tag="xsb")
    xt_all = const.tile([P, NT * F], BF16, tag="xta")
    for mu in range(NT):
        nc.sync.dma_start(
            out=xs_f32[:, mu * F : (mu + 1) * F], in_=X[mu * P : (mu + 1) * P, :]
        )
        nc.gpsimd.tensor_copy(
            xs_bf[:, mu * F : (mu + 1) * F], xs_f32[:, mu * F : (mu + 1) * F]
        )
    # xt_all[p, 128*(2*mu + phi) + r] = X[128*mu + r, 128*phi + p]
    nc.sync.dma_start_transpose(
        xt_all[:, :].rearrange("p (e r) -> p e r", e=2 * NT), xs_bf[:, :]
    )
    for mu in range(NT):
        t_psum = psum_acc.tile([P, F], F32, tag="acc", name="t_psum")
        for phi in range(FT):
            nc.tensor.matmul(
                t_psum[:, :],
                xt_all[:, (2 * mu + phi) * P : (2 * mu + phi + 1) * P],
                wt_sb[:, phi * F : (phi + 1) * F],
                start=(phi == 0),
                stop=(phi == FT - 1),
            )
        # t + b -> t_big f32 (DVE, PSUM read)
        nc.vector.tensor_add(t_big[:, mu * F : (mu + 1) * F], t_psum[:, :], b_bcast[:, :])

    # ---- stream A row-tiles: degree, d, y, xbar transpose, main matmul ----
    acc_tiles = {}

    def emit_product(k, mu):
        nc.tensor.matmul(
            acc_tiles[mu][:, :],
            at_view[:, mu, k, :],
            y_big[:, k * F : (k + 1) * F],
            start=(k == 0),
            stop=(k == NT - 1),
        )

    def emit_drain(mu):
        os = outstage.tile([P, F], F32, tag="os")
        nc.scalar.activation(
            os[:, :], acc_tiles[mu][:, :], RELU, scale=dinv[:, mu : mu + 1]
        )
        nc.gpsimd.dma_start(out=OUT[mu * P : (mu + 1) * P, :], in_=os[:, :])

    PREFETCH = 3
    a_f32_tiles = {}

    def emit_load(j):
        a_f32_tiles[j] = stage.tile([P, N], F32, tag="af", name=f"a_f32_{j}")
        nc.sync.dma_start(out=a_f32_tiles[j][:, :], in_=A[j * P : (j + 1) * P, :])

    for j in range(PREFETCH):
        emit_load(j)

    for i in range(NT):
        if i + PREFETCH < NT:
            emit_load(i + PREFETCH)
        a_f32 = a_f32_tiles.pop(i)
        # one DVE pass: bf16 cast (matmul operand) + row-sum degree accumulator
        a_bf = stage.tile([P, N], BF16, tag="a")
        nc.vector.tensor_scalar(
            out=a_bf[:, :],
            in0=a_f32[:, :],
            scalar1=0.0,
            scalar2=None,
            op0=mybir.AluOpType.add,
            op1=mybir.AluOpType.add,
            accum_out=deg[:, i : i + 1],
        )
        nc.vector.reciprocal(rec[:, i : i + 1], deg[:, i : i + 1])
        nc.scalar.sqrt(dinv[:, i : i + 1], rec[:, i : i + 1])
        # y[i] = d[i] * t[i], rounded to bf16
        nc.scalar.activation(
            y_big[:, i * F : (i + 1) * F],
            t_big[:, i * F : (i + 1) * F],
            COPY,
            scale=dinv[:, i : i + 1],
        )
        # xbar-transpose this row-tile into at_big (3D out: chunk-contiguous)
        nc.sync.dma_start_transpose(at_view[:, i, :, :], a_bf[:, :])
        # main-matmul products that just became runnable (early accumulators):
        # every (k, mu) pair with max(k, mu) == i and mu < ACC_SLOTS
        if i < ACC_SLOTS:
            acc_tiles[i] = psum_acc.tile([P, F], F32, tag="acc", name=f"acc_{i}")
            for k in range(i + 1):
                emit_product(k, i)
        for mu in range(min(i, ACC_SLOTS)):
            emit_product(i, mu)

    # ---- drains + tail batches ----
    for mu in range(ACC_SLOTS):
        emit_drain(mu)
    for mu in range(ACC_SLOTS, NT):
        acc_tiles[mu] = psum_acc.tile([P, F], F32, tag="acc", name=f"acc_{mu}")
        for k in range(NT):
            emit_product(k, mu)
        emit_drain(mu)


_cached_nc = None


def _build():
    nc = bacc.Bacc("TRN2", target_bir_lowering=False, debug=False)
    A = nc.dram_tensor("adj", [N, N], F32, kind="ExternalInput").ap()
    X = nc.dram_tensor("x", [N, F], F32, kind="ExternalInput").ap()
    WT = nc.dram_tensor("wt", [F, F], F32, kind="ExternalInput").ap()
    BIAS = nc.dram_tensor("bias", [1, F], F32, kind="ExternalInput").ap()
    OUT = nc.dram_tensor("out", [N, F], F32, kind="ExternalOutput").ap()
    with tile.TileContext(nc) as tc:
        with ExitStack() as ctx:
            _emit(ctx, tc, A, X, WT, BIAS, OUT)
    nc.compile()
    return nc


def get_nc():
    global _cached_nc
    if _cached_nc is None:
        _cached_nc = _build()
    return _cached_nc


def make_in_maps(node_features, adj_matrix, W, b):
    node_features = np.asarray(node_features, dtype=np.float32)
    adj_matrix = np.asarray(adj_matrix, dtype=np.float32)
    wt = np.ascontiguousarray(np.asarray(W, dtype=np.float32).T)
    bias = np.ascontiguousarray(np.asarray(b, dtype=np.float32).reshape(1, F))
    return [
        {
            "adj": np.ascontiguousarray(adj_matrix[c]),
            "x": np.ascontiguousarray(node_features[c]),
            "wt": wt,
            "bias": bias,
        }
        for c in range(B)
    ]


def kernel(node_features, adj_matrix, W, b):
    nc = get_nc()
    in_maps = make_in_maps(node_features, adj_matrix, W, b)
    res = run_bass_kernel_spmd(nc, in_maps, core_ids=list(range(B)))
    return np.stack([r["out"] for r in res.results], axis=0)
